# revision 30
# baseline (speedup 1.0000x reference)
"""Trainium2 Bass kernel for nn_CustomTransformer_58445914964311.

12-layer MoE transformer (768 embd, 8 heads, 8 experts top-2, B=8 x T=64
tokens), distributed over 8 NeuronCores:
  - attention sharded by head (core c computes head c for all 512 tokens),
  - MoE sharded by expert (core c computes expert c densely for all tokens,
    weighted by its combine weight),
  - per-layer AllReduce combines the per-head attention partials and the
    per-expert MoE partials; everything else is computed replicated.

All activations are kept feature-major in SBUF ([features-on-partitions,
tokens-on-free]) so linear layers chain on the PE without transposes.

Precision scheme: the top-2 gate decisions are tie-sensitive (min prob gap
~2.5e-6), so everything feeding a gate must track the fp32 trajectory to
~2^-20. Plain fp16/bf16/float32r matmuls flip gates (whole-token errors).
Instead the heavy matmuls (MoE w1/w2, QKV, proj, LN sums) use an exact
fp16 triple-pass split: W = Wh + Wl/LSCALE, X = Xh + Xl, computing
Wh*Xh + Wh*Xl + (Wl)*(Xh/LSCALE) with fp32 PSUM accumulation. Each pass
streams at 1 cycle/row (vs 4 for fp32), giving ~2^-21 error: 3 cycles/row
total with zero gate flips (33x margin in simulation). The last layer's
MoE feeds no gate and runs single-pass fp16. Gate/softmax/layernorm
arithmetic stays fp32. AllReduces are fp32 and split in two halves so the
second half overlaps the tail of the producing matmuls.

Self-contained: hardcodes all shapes; host side only reshapes/transposes
and shards the incoming fp32 weights into fp16 hi/lo pairs.
"""

import numpy as np

import concourse.bass as bass
import concourse.mybir as mybir
import concourse.tile as tile
from concourse.bass_utils import run_bass_kernel_spmd

import os
import sys

# ---------------------------------------------------------------------------
# Compatibility patches (inlined): the walrus build here rejects instructions
# carrying more than one semaphore wait ("Too many sync wait commands").
# 1) split the Tile kernel-tail drain's waits onto separate sync nops;
# 2) post-process the serialized BIR, peeling extra waits onto injected
#    EventSemaphore instructions;
# 3) recreate the missing antenv.axon_hooks registry so trace=True works.
# ---------------------------------------------------------------------------
import orjson as _orjson
from concourse.vector_clock import ScopedClock as _ScopedClock

_COMPAT_DONE = False


def _patched_drain_and_barrier(self, tick_clock, wait_clock):
    nc = self.nc
    collector = nc.sync.nop()
    wait_clock.add_sem_waits(
        collector.ins, _ScopedClock({None: tick_clock.global_clock})
    )
    si = collector.ins.sync_info
    waits = list(si.on_wait or []) if si is not None else []
    if len(waits) > 1:
        si.on_wait = waits[:1]
        for w in waits[1:]:
            extra = nc.sync.nop()
            esi = extra.ins.sync_info
            if esi is None:
                extra.ins.sync_info = mybir.SyncInfo(on_wait=[w], on_update=[])
            else:
                esi.on_wait = [w]
    nc.sync.drain()
    nc.all_engine_barrier()
    popped = nc._tile_sem_poison_stack.pop()
    assert popped is self._sem_poison
    nc.clear_and_free_semaphores(list(self.sems.allocated().values()))
    nc.all_engine_barrier()


def _split_multi_waits(mod, max_waits=1):
    ctr = 0
    for fn in mod.get("functions", []):
        for blk in fn.get("blocks", []):
            insts = blk.get("instructions", [])
            if not any(
                len((i.get("sync_info") or {}).get("on_wait") or []) > max_waits
                for i in insts
            ):
                continue
            new_insts = []
            for inst in insts:
                si = inst.get("sync_info")
                waits = (si.get("on_wait") or []) if si else []
                if len(waits) > max_waits:
                    for w in waits[max_waits:]:
                        ctr += 1
                        new_insts.append({
                            "debug": inst.get("debug", 0),
                            "engine": inst["engine"],
                            "ins": [], "outs": [],
                            "name": f"{inst['name']}-wsp{ctr}",
                            "opcode": "EventSemaphore",
                            "sync_info": {"on_update": [], "on_wait": [w]},
                        })
                    si["on_wait"] = waits[:max_waits]
                new_insts.append(inst)
            blk["instructions"] = new_insts
    return mod


_orig_to_json_bytes = bass.Bass.to_json_bytes


def _patched_to_json_bytes(self):
    return _orjson.dumps(_split_multi_waits(_orjson.loads(_orig_to_json_bytes(self))))


def _install_ntff_hook_shim():
    import types
    if "antenv.axon_hooks" in sys.modules:
        return
    try:
        import antenv  # noqa: F401
    except ImportError:
        return
    mod = types.ModuleType("antenv.axon_hooks")
    _state = {"hook": None}
    mod.set_axon_ntff_profile_hook = lambda hook: _state.__setitem__("hook", hook)
    mod.get_axon_ntff_profile_hook = lambda: _state["hook"]
    sys.modules["antenv.axon_hooks"] = mod
    sys.modules["antenv"].axon_hooks = mod
    try:
        from trn_agent_boot.trn_boot import _ntff_profile_via_ctypes
        hook = _ntff_profile_via_ctypes("/opt/axon/libaxon_pjrt.so")
        if hook is not None:
            mod.set_axon_ntff_profile_hook(hook)
    except Exception:
        pass


def _install_compat():
    global _COMPAT_DONE
    if _COMPAT_DONE:
        return
    tile.TileContext._drain_and_barrier = _patched_drain_and_barrier
    bass.Bass.to_json_bytes = _patched_to_json_bytes
    _install_ntff_hook_shim()
    _COMPAT_DONE = True


_install_compat()

F32 = mybir.dt.float32
F16 = mybir.dt.float16
BF16 = mybir.dt.bfloat16
I32 = mybir.dt.int32
AF = mybir.ActivationFunctionType
ALU = mybir.AluOpType
AX = mybir.AxisListType
LSCALE = 1024.0  # scaling of the low fp16 half (keeps it in normal range)

N_CORES = 8
L = 12
D = 768
H = 96          # head dim
NH = 8
E = 8           # experts
DFF = 3072
B, T = 8, 64
N = B * T       # 512 tokens
V = 99
KT = D // 128   # 6 feature tiles
MT = DFF // 128  # 24 dff tiles
L1 = L - 1      # layers with fp16x3 MoE (last layer runs bf16)
EPS = 1e-5
SCALE = H ** -0.5

_CACHED = {}


def build():
    nc = bass.Bass(num_devices=N_CORES)

    # ---- inputs (per-core data, same names) ----
    d_idx = nc.dram_tensor("idx", [1, N], I32, kind="ExternalInput")
    d_iota = nc.dram_tensor("iota99", [V, 1], F32, kind="ExternalInput")
    d_ident = nc.dram_tensor("ident128", [128, 128], F32, kind="ExternalInput")
    d_mask = nc.dram_tensor("maskb", [64, 64], F32, kind="ExternalInput")
    d_ones_col = nc.dram_tensor("ones_col", [128, 1], F32, kind="ExternalInput")
    d_ones16 = nc.dram_tensor("ones16", [128, 1], F16, kind="ExternalInput")
    d_ones_row = nc.dram_tensor("ones_row", [1, 128], F32, kind="ExternalInput")
    d_tok = nc.dram_tensor("tok_emb", [V, D], F32, kind="ExternalInput")
    d_posT = nc.dram_tensor("posT", [D, N], F32, kind="ExternalInput")
    d_wqp = nc.dram_tensor("wqp", [L, 2, KT, 128, H], F16, kind="ExternalInput")
    d_wkp = nc.dram_tensor("wkp", [L, 2, KT, 128, H], F16, kind="ExternalInput")
    d_wvp = nc.dram_tensor("wvp", [L, 2, KT, 128, H], F16, kind="ExternalInput")
    d_wpp = nc.dram_tensor("wpp", [L, 2, H, D], F16, kind="ExternalInput")
    d_bproj = nc.dram_tensor("bproj", [L, KT, 128], F32, kind="ExternalInput")
    d_gwT = nc.dram_tensor("gwT", [L, KT, 128, E], F32, kind="ExternalInput")
    d_gb = nc.dram_tensor("gb", [L, 1, E], F32, kind="ExternalInput")
    d_w1p = nc.dram_tensor("w1p", [L1, MT, 2, KT, 128, 128], F16,
                           kind="ExternalInput")
    d_b1 = nc.dram_tensor("b1", [L, MT, 128], F32, kind="ExternalInput")
    d_w2p = nc.dram_tensor("w2p", [L1, KT, 2, 2, 12, 128, 128], F16,
                           kind="ExternalInput")
    d_w1b = nc.dram_tensor("w1b", [MT, KT, 128, 128], F16, kind="ExternalInput")
    d_w2b = nc.dram_tensor("w2b", [KT, MT, 128, 128], F16, kind="ExternalInput")
    d_b2 = nc.dram_tensor("b2all", [L, E, D], F32, kind="ExternalInput")
    d_combsel = nc.dram_tensor("combsel", [E, 1], F32, kind="ExternalInput")
    d_ln1w = nc.dram_tensor("ln1w", [L, KT, 128], F32, kind="ExternalInput")
    d_ln1b = nc.dram_tensor("ln1b", [L, KT, 128], F32, kind="ExternalInput")
    d_ln2w = nc.dram_tensor("ln2w", [L, KT, 128], F32, kind="ExternalInput")
    d_ln2b = nc.dram_tensor("ln2b", [L, KT, 128], F32, kind="ExternalInput")
    d_lnfw = nc.dram_tensor("lnfw", [KT, 128], F32, kind="ExternalInput")
    d_lnfb = nc.dram_tensor("lnfb", [KT, 128], F32, kind="ExternalInput")
    d_lmT = nc.dram_tensor("lmT", [KT, 128, V], F32, kind="ExternalInput")
    d_lmb = nc.dram_tensor("lmb", [V, 1], F32, kind="ExternalInput")
    d_out = nc.dram_tensor("logitsT", [V, N], F32, kind="ExternalOutput")

    with tile.TileContext(nc) as tc:
        with (
            tc.tile_pool(name="const", bufs=1) as cpool,
            tc.tile_pool(name="x", bufs=1) as xpool,
            tc.tile_pool(name="attw", bufs=1) as awpool,
            tc.tile_pool(name="w1", bufs=3) as w1pool,
            tc.tile_pool(name="w2", bufs=2) as w2pool,
            tc.tile_pool(name="h", bufs=2) as hpool,
            tc.tile_pool(name="hsp", bufs=1) as hsp,
            tc.tile_pool(name="xsp", bufs=1) as xsp,
            tc.tile_pool(name="work", bufs=2) as wk,
            tc.tile_pool(name="small", bufs=3) as sm,
            tc.tile_pool(name="ps_acc", bufs=3, space="PSUM") as ps_acc,
            tc.tile_pool(name="ps_small", bufs=3, space="PSUM") as ps_small,
            tc.tile_pool(name="ps_bc", bufs=2, space="PSUM") as ps_bc,
            tc.tile_pool(name="dram", bufs=1, space="DRAM") as dpool,
        ):
            # ---- constants resident ----
            ident = cpool.tile([128, 128], F32, name="ident")
            nc.sync.dma_start(ident[:], d_ident[:])
            maskb = cpool.tile([64, 64], F32, name="maskb")
            nc.sync.dma_start(maskb[:], d_mask[:])
            iota99 = cpool.tile([V, 1], F32, name="iota99")
            nc.sync.dma_start(iota99[:], d_iota[:])
            ones_col = cpool.tile([128, 1], F32, name="ones_col")
            nc.sync.dma_start(ones_col[:], d_ones_col[:])
            ones16 = cpool.tile([128, 1], F16, name="ones16")
            nc.sync.dma_start(ones16[:], d_ones16[:])
            ones_row = cpool.tile([1, 128], F32, name="ones_row")
            nc.sync.dma_start(ones_row[:], d_ones_row[:])
            combsel = cpool.tile([E, 1], F32, name="combsel")
            nc.sync.dma_start(combsel[:], d_combsel[:])
            tok = cpool.tile([V, D], F32, name="tok")
            nc.sync.dma_start(tok[:], d_tok[:])
            posT = wk.tile([128, KT * N], F32, name="ln_t", bufs=1)
            for k in range(KT):
                nc.sync.dma_start(posT[:, k * N:(k + 1) * N],
                                  d_posT[k * 128:(k + 1) * 128, :])
            lmT = cpool.tile([128, KT * V], F32, name="lmT")
            for k in range(KT):
                nc.sync.dma_start(lmT[:, k * V:(k + 1) * V], d_lmT[k])
            lmb = cpool.tile([V, 1], F32, name="lmb")
            nc.sync.dma_start(lmb[:], d_lmb[:])
            lnfw = cpool.tile([128, KT], F32, name="lnfw")
            nc.sync.dma_start(lnfw[:], d_lnfw.rearrange("a p -> p a"))
            lnfb = cpool.tile([128, KT], F32, name="lnfb")
            nc.sync.dma_start(lnfb[:], d_lnfb.rearrange("a p -> p a"))

            # AR bounce tensors, one per (layer, attn/moe, half of 3 ktiles)
            ar_ins = [dpool.tile([384, N], F32, name=f"ari{i}")
                      for i in range(4 * L)]
            ar_outs = [dpool.tile([384, N], F32, name=f"aro{i}",
                                  addr_space="Shared") for i in range(4 * L)]

            def run_ar(idx):
                nc.gpsimd.collective_compute(
                    "AllReduce", ALU.add,
                    replica_groups=[list(range(N_CORES))],
                    ins=[ar_ins[idx][:]], outs=[ar_outs[idx][:]])

            def split3(src_ap, pool, tag, p_dim, n_cols, scaled=True, bufs=1):
                """fp16 triple (hi, lo, hi/LSCALE) of a fp32 AP."""
                hi = pool.tile([p_dim, n_cols], F16, tag=tag + "h", bufs=bufs)
                nc.vector.tensor_copy(hi[:], src_ap)
                lo = pool.tile([p_dim, n_cols], F16, tag=tag + "l", bufs=bufs)
                nc.vector.scalar_tensor_tensor(lo[:], src_ap, 1.0, hi[:],
                                               op0=ALU.mult, op1=ALU.subtract)
                if not scaled:
                    return hi, lo, None
                hs = pool.tile([p_dim, n_cols], F16, tag=tag + "s", bufs=bufs)
                nc.vector.tensor_scalar_mul(hs[:], hi[:], 1.0 / LSCALE)
                return hi, lo, hs

            # ---- x state: 6 tiles [128, N] ----
            x_sb = xpool.tile([128, KT * N], F32, name="x_sb")

            def xs(k):
                return x_sb[:, k * N:(k + 1) * N]

            # ---- embedding ----
            idx_i = sm.tile([1, N], I32, name="idx_i", bufs=1)
            nc.sync.dma_start(idx_i[:], d_idx[:])
            idx_f = sm.tile([1, N], F32, name="idx_f", bufs=1)
            nc.vector.tensor_copy(idx_f[:], idx_i[:])
            idxbc = ps_bc.tile([V, N], F32, tag="bc")
            nc.tensor.matmul(idxbc[:], ones_row[:, :V], idx_f[:],
                             start=True, stop=True)
            onehot = wk.tile([V, N], F32, name="onehot", bufs=1)
            nc.vector.tensor_scalar(onehot[:], idxbc[:], iota99[:], None,
                                    op0=ALU.is_equal)
            for k in range(KT):
                e_ps = ps_acc.tile([128, N], F32, tag="acc")
                nc.tensor.matmul(e_ps[:], tok[:, k * 128:(k + 1) * 128],
                                 onehot[:], start=True, stop=True)
                nc.vector.tensor_add(xs(k), e_ps[:], posT[:, k * N:(k + 1) * N])

            def layernorm(get_t, w_ap, b_ap, extra_ps=None):
                """get_t(k) -> [128, N] AP of pre-norm values (read twice).
                Writes normalized result into x_sb. extra_ps: optional psum
                [128,N]-AP-producing fn added into t (used for b2@combT)."""
                s_ps = ps_small.tile([1, N], F32, tag="sm")
                q_ps = ps_small.tile([1, N], F32, tag="sm")
                tmp = wk.tile([128, KT * N], F32, name="ln_t", bufs=1)
                for k in range(KT):
                    tk = tmp[:, k * N:(k + 1) * N]
                    src = get_t(k)
                    if extra_ps is not None:
                        nc.vector.tensor_add(tk, src, extra_ps(k))
                        src = tk
                    else:
                        nc.vector.tensor_copy(tk, src)
                    th = sm.tile([128, N], F16, tag="lnth", bufs=1)
                    nc.vector.tensor_copy(th[:], tk)
                    tl = sm.tile([128, N], F16, tag="lntl", bufs=1)
                    nc.vector.scalar_tensor_tensor(tl[:], tk, 1.0, th[:],
                                                   op0=ALU.mult,
                                                   op1=ALU.subtract)
                    sq = sm.tile([128, N], F32, tag="lnsq", bufs=1)
                    nc.scalar.activation(sq[:], tk, AF.Square)
                    sqh = sm.tile([128, N], F16, tag="lnsqh", bufs=1)
                    nc.vector.tensor_copy(sqh[:], sq[:])
                    sql = sm.tile([128, N], F16, tag="lnsql", bufs=1)
                    nc.vector.scalar_tensor_tensor(sql[:], sq[:], 1.0, sqh[:],
                                                   op0=ALU.mult,
                                                   op1=ALU.subtract)
                    nc.tensor.matmul(s_ps[:], ones16[:], th[:],
                                     start=(k == 0), stop=False)
                    nc.tensor.matmul(s_ps[:], ones16[:], tl[:],
                                     start=False, stop=(k == KT - 1))
                    nc.tensor.matmul(q_ps[:], ones16[:], sqh[:],
                                     start=(k == 0), stop=False)
                    nc.tensor.matmul(q_ps[:], ones16[:], sql[:],
                                     start=False, stop=(k == KT - 1))
                mu = sm.tile([1, N], F32, tag="ln1", bufs=1)
                nc.vector.tensor_scalar_mul(mu[:], s_ps[:], 1.0 / D)
                mu2 = sm.tile([1, N], F32, tag="ln2", bufs=1)
                nc.vector.tensor_mul(mu2[:], mu[:], mu[:])
                var = sm.tile([1, N], F32, tag="ln3", bufs=1)
                nc.vector.scalar_tensor_tensor(var[:], q_ps[:], 1.0 / D, mu2[:],
                                               op0=ALU.mult, op1=ALU.subtract)
                nc.vector.tensor_scalar_add(var[:], var[:], EPS)
                sd = sm.tile([1, N], F32, tag="ln4", bufs=1)
                nc.scalar.activation(sd[:], var[:], AF.Sqrt)
                rstd = sm.tile([1, N], F32, tag="ln5", bufs=1)
                nc.vector.reciprocal(rstd[:], sd[:])
                nmu = sm.tile([1, N], F32, tag="ln6", bufs=1)
                nc.vector.tensor_scalar_mul(nmu[:], mu[:], -1.0)
                nmu_bc = ps_bc.tile([128, N], F32, tag="bc")
                nc.tensor.matmul(nmu_bc[:], ones_row[:], nmu[:],
                                 start=True, stop=True)
                rstd_bc = ps_bc.tile([128, N], F32, tag="bc")
                nc.tensor.matmul(rstd_bc[:], ones_row[:], rstd[:],
                                 start=True, stop=True)
                for k in range(KT):
                    tk = tmp[:, k * N:(k + 1) * N]
                    u = sm.tile([128, N], F32, tag="lnu", bufs=1)
                    nc.vector.tensor_add(u[:], tk, nmu_bc[:])
                    nc.vector.tensor_mul(u[:], u[:], rstd_bc[:])
                    nc.vector.tensor_scalar(xs(k), u[:], w_ap[:, k:k + 1],
                                            b_ap[:, k:k + 1],
                                            op0=ALU.mult, op1=ALU.add)

            for l in range(L):
                # ---- layer weights (fp16 hi/lo pairs) ----
                wq = awpool.tile([128, 2 * KT * H], F16, tag="wq")
                nc.sync.dma_start(
                    wq[:].rearrange("q (p k h) -> q p k h", p=2, k=KT),
                    d_wqp[l].rearrange("p k q h -> q p k h"))
                wkk = awpool.tile([128, 2 * KT * H], F16, tag="wk")
                nc.sync.dma_start(
                    wkk[:].rearrange("q (p k h) -> q p k h", p=2, k=KT),
                    d_wkp[l].rearrange("p k q h -> q p k h"))
                wv = awpool.tile([128, 2 * KT * H], F16, tag="wv")
                nc.sync.dma_start(
                    wv[:].rearrange("q (p k h) -> q p k h", p=2, k=KT),
                    d_wvp[l].rearrange("p k q h -> q p k h"))
                wp = awpool.tile([H, 2 * D], F16, tag="wp")
                nc.sync.dma_start(
                    wp[:].rearrange("h (p d) -> h p d", p=2),
                    d_wpp[l].rearrange("p h d -> h p d"))
                bpj = awpool.tile([128, KT], F32, tag="bpj")
                nc.sync.dma_start(bpj[:], d_bproj[l].rearrange("a p -> p a"))
                gw = awpool.tile([128, KT * E], F32, tag="gw")
                for k in range(KT):
                    nc.sync.dma_start(gw[:, k * E:(k + 1) * E], d_gwT[l, k])
                gb = awpool.tile([1, E], F32, tag="gb")
                nc.sync.dma_start(gb[:], d_gb[l])
                l1w = awpool.tile([128, KT], F32, tag="l1w")
                nc.sync.dma_start(l1w[:], d_ln1w[l].rearrange("a p -> p a"))
                l1b = awpool.tile([128, KT], F32, tag="l1b")
                nc.sync.dma_start(l1b[:], d_ln1b[l].rearrange("a p -> p a"))
                l2w = awpool.tile([128, KT], F32, tag="l2w")
                nc.sync.dma_start(l2w[:], d_ln2w[l].rearrange("a p -> p a"))
                l2b = awpool.tile([128, KT], F32, tag="l2b")
                nc.sync.dma_start(l2b[:], d_ln2b[l].rearrange("a p -> p a"))
                b1t = awpool.tile([128, MT], F32, tag="b1t")
                nc.sync.dma_start(b1t[:], d_b1[l].rearrange("a p -> p a"))
                b2t = awpool.tile([E, D], F32, tag="b2t")
                nc.sync.dma_start(b2t[:], d_b2[l])

                # ---- fp16 split of layer-input x (for QKV) ----
                xa_h, xa_l, xa_s = split3(x_sb[:], xsp, "xa", 128, KT * N)

                def x3mm(ps, w_pair, wtile_cols, n_mm=KT, x_trip=None):
                    """18 accumulating matmuls: Wh*Xh + Wh*Xl + Wl'*Xs."""
                    trip = x_trip or (xa_h, xa_l, xa_s)
                    i = 0
                    for wp_, xt in ((0, trip[0]), (0, trip[1]), (1, trip[2])):
                        for k in range(n_mm):
                            c0 = (wp_ * n_mm + k) * wtile_cols
                            nc.tensor.matmul(
                                ps, w_pair[:, c0:c0 + wtile_cols],
                                xt[:, k * N:(k + 1) * N],
                                start=(i == 0), stop=(i == 3 * n_mm - 1))
                            i += 1

                # ---- attention: this core's head ----
                q_ps = ps_acc.tile([H, N], F32, tag="acc")
                k_ps = ps_acc.tile([H, N], F32, tag="acc")
                v_ps = ps_acc.tile([H, N], F32, tag="acc")
                x3mm(q_ps[:], wq, H)
                x3mm(k_ps[:], wkk, H)
                x3mm(v_ps[:], wv, H)
                qT = wk.tile([H, N], F32, name="qT", bufs=1)
                kT_ = wk.tile([H, N], F32, name="kT", bufs=1)
                vT = wk.tile([H, N], F32, name="vT", bufs=1)
                nc.vector.tensor_copy(qT[:], q_ps[:])
                nc.vector.tensor_copy(kT_[:], k_ps[:])
                nc.vector.tensor_copy(vT[:], v_ps[:])

                oT = wk.tile([H, N], F32, name="oT", bufs=1)
                for b in range(B):
                    ts_ = slice(b * 64, (b + 1) * 64)
                    w_ps = ps_small.tile([64, 64], F32, tag="sm")
                    nc.tensor.matmul(w_ps[:], qT[:, ts_], kT_[:, ts_],
                                     start=True, stop=True)
                    s_sb = sm.tile([64, 64], F32, tag="att_s")
                    nc.vector.scalar_tensor_tensor(s_sb[:], w_ps[:], SCALE,
                                                   maskb[:], op0=ALU.mult,
                                                   op1=ALU.add)
                    mx = sm.tile([64, 1], F32, tag="att_m")
                    nc.vector.reduce_max(mx[:], s_sb[:], axis=AX.X, negate=True)
                    att = sm.tile([64, 64], F32, tag="att_a")
                    ssum = sm.tile([64, 1], F32, tag="att_su")
                    nc.scalar.activation(att[:], s_sb[:], AF.Exp, bias=mx[:],
                                         accum_out=ssum[:])
                    rs = sm.tile([64, 1], F32, tag="att_r")
                    nc.vector.reciprocal(rs[:], ssum[:])
                    nc.vector.tensor_scalar_mul(att[:], att[:], rs[:])
                    at_ps = ps_small.tile([64, 64], F32, tag="sm")
                    nc.tensor.transpose(at_ps[:], att[:], ident[:64, :64])
                    attT = sm.tile([64, 64], F32, tag="att_t")
                    nc.vector.tensor_copy(attT[:], at_ps[:])
                    vt_ps = ps_small.tile([64, H], F32, tag="sm")
                    nc.tensor.transpose(vt_ps[:], vT[:, ts_], ident[:H, :H])
                    vtb = sm.tile([64, H], F32, tag="att_v")
                    nc.vector.tensor_copy(vtb[:], vt_ps[:])
                    o_ps = ps_small.tile([H, 64], F32, tag="sm")
                    nc.tensor.matmul(o_ps[:], vtb[:], attT[:],
                                     start=True, stop=True)
                    nc.vector.tensor_copy(oT[:, ts_], o_ps[:])

                # proj partials -> ar_in (fp16x3, AR split in 2 halves)
                o_h, o_l, o_s = split3(oT[:], xsp, "osp", H, N)
                for m in range(KT):
                    y_ps = ps_acc.tile([128, N], F32, tag="acc")
                    for i, (wcol, ot) in enumerate((
                            (m * 128, o_h), (m * 128, o_l),
                            (D + m * 128, o_s))):
                        nc.tensor.matmul(y_ps[:], wp[:, wcol:wcol + 128],
                                         ot[:], start=(i == 0), stop=(i == 2))
                    yc = sm.tile([128, N], F32, tag="ycp", bufs=2)
                    nc.vector.tensor_copy(yc[:], y_ps[:])
                    nc.sync.dma_start(
                        ar_ins[4 * l + m // 3][(m % 3) * 128:
                                               (m % 3 + 1) * 128, :], yc[:])
                    if m == 2:
                        run_ar(4 * l)
                run_ar(4 * l + 1)
                yat = wk.tile([128, KT * N], F32, name="yat", bufs=1)
                for k in range(KT):
                    nc.sync.dma_start(
                        yat[:, k * N:(k + 1) * N],
                        ar_outs[4 * l + k // 3][(k % 3) * 128:
                                                (k % 3 + 1) * 128, :])

                # residual + bproj + ln1  (t = (y + bproj) + x)
                def get_t1(k, yat=yat, bpj=bpj):
                    u = sm.tile([128, N], F32, tag="res_u", bufs=2)
                    nc.vector.scalar_tensor_tensor(
                        u[:], yat[:, k * N:(k + 1) * N], bpj[:, k:k + 1],
                        xs(k), op0=ALU.add, op1=ALU.add)
                    return u[:]

                layernorm(get_t1, l1w, l1b)

                # ---- gate + top2 comb ----
                combT = sm.tile([E, N], F32, tag="combT", bufs=1)
                for tt in range(4):
                    g_ps = ps_small.tile([128, E], F32, tag="sm")
                    for k in range(KT):
                        nc.tensor.matmul(
                            g_ps[:],
                            x_sb[:, k * N + tt * 128:k * N + (tt + 1) * 128],
                            gw[:, k * E:(k + 1) * E],
                            start=(k == 0), stop=False)
                    nc.tensor.matmul(g_ps[:], ones_row[:], gb[:],
                                     start=False, stop=True)
                    gl = sm.tile([128, E], F32, tag="g_l")
                    mx = sm.tile([128, 1], F32, tag="g_m")
                    nc.vector.reduce_max(mx[:], g_ps[:], axis=AX.X, negate=True)
                    pr = sm.tile([128, E], F32, tag="g_p")
                    ssum = sm.tile([128, 1], F32, tag="g_s")
                    nc.scalar.activation(pr[:], g_ps[:], AF.Exp, bias=mx[:],
                                         accum_out=ssum[:])
                    rs = sm.tile([128, 1], F32, tag="g_r")
                    nc.vector.reciprocal(rs[:], ssum[:])
                    nc.vector.tensor_scalar_mul(pr[:], pr[:], rs[:])
                    top8 = sm.tile([128, 8], F32, tag="g_t8")
                    nc.vector.max(out=top8[:], in_=pr[:])
                    msk = sm.tile([128, E], F32, tag="g_msk")
                    nc.vector.tensor_scalar(msk[:], pr[:], top8[:, 1:2], None,
                                            op0=ALU.is_ge)
                    cw = sm.tile([128, E], F32, tag="g_cw")
                    nc.vector.tensor_mul(cw[:], pr[:], msk[:])
                    den = sm.tile([128, 1], F32, tag="g_den")
                    nc.vector.tensor_add(den[:], top8[:, 0:1], top8[:, 1:2])
                    dr = sm.tile([128, 1], F32, tag="g_dr")
                    nc.vector.reciprocal(dr[:], den[:])
                    nc.vector.tensor_scalar_mul(cw[:], cw[:], dr[:])
                    ct_ps = ps_small.tile([E, 128], F32, tag="sm")
                    nc.tensor.transpose(ct_ps[:], cw[:], ident[:])
                    nc.vector.tensor_copy(combT[:, tt * 128:(tt + 1) * 128],
                                          ct_ps[:])
                # this core's expert row -> broadcast [128, N]
                crow_ps = ps_small.tile([1, N], F32, tag="sm")
                nc.tensor.matmul(crow_ps[:], combsel[:], combT[:],
                                 start=True, stop=True)
                crow = sm.tile([1, N], F32, tag="crow", bufs=1)
                nc.vector.tensor_copy(crow[:], crow_ps[:])
                cbc_ps = ps_bc.tile([128, N], F32, tag="bc")
                nc.tensor.matmul(cbc_ps[:], ones_row[:], crow[:],
                                 start=True, stop=True)
                cbc = wk.tile([128, N], F32, name="cbc", bufs=1)
                nc.vector.tensor_copy(cbc[:], cbc_ps[:])

                # ---- MoE expert (dense over all tokens) ----
                if l < L - 1:
                    # fp16x3 exact-split path (reuses the QKV split tiles --
                    # attention consumed them before ln1 completed), processed
                    # in two DFF halves to bound the h-split SBUF footprint
                    x1t = split3(x_sb[:], xsp, "xa", 128, KT * N)
                    hh = hsp.tile([128, 12 * N], F16, tag="hh")
                    hl = hsp.tile([128, 12 * N], F16, tag="hl")
                    hs = hsp.tile([128, 12 * N], F16, tag="hs")
                    yeh = wk.tile([128, KT * N], F32, name="yeh", bufs=1)
                    for dh in range(2):
                        for mm_ in range(12):
                            m = dh * 12 + mm_
                            w1m = w1pool.tile([128, 2 * KT * 128], F16,
                                              tag="w1", bufs=2)
                            nc.sync.dma_start(
                                w1m[:].rearrange("q (p k f) -> q p k f",
                                                 p=2, k=KT),
                                d_w1p[l, m].rearrange("p k q f -> q p k f"))
                            h_ps = ps_acc.tile([128, N], F32, tag="acc")
                            x3mm(h_ps[:], w1m, 128, x_trip=x1t)
                            ht = hpool.tile([128, N], F32, tag="ht")
                            nc.scalar.activation(ht[:], h_ps[:], AF.Gelu,
                                                 bias=b1t[:, m:m + 1])
                            ms = slice(mm_ * N, (mm_ + 1) * N)
                            nc.vector.tensor_copy(hh[:, ms], ht[:])
                            nc.vector.scalar_tensor_tensor(
                                hl[:, ms], ht[:], 1.0, hh[:, ms],
                                op0=ALU.mult, op1=ALU.subtract)
                            nc.vector.tensor_scalar_mul(hs[:, ms], hh[:, ms],
                                                        1.0 / LSCALE)
                        for mo in range(KT):
                            w2m = w2pool.tile([128, 2 * 12 * 128], F16,
                                              tag="w2")
                            nc.sync.dma_start(
                                w2m[:].rearrange("q (p j f) -> q p j f",
                                                 p=2, j=12),
                                d_w2p[l, mo, dh]
                                .rearrange("p j q f -> q p j f"))
                            ye_ps = ps_acc.tile([128, N], F32, tag="acc")
                            i = 0
                            for wp_, ht_ in ((0, hh), (0, hl), (1, hs)):
                                for jj in range(12):
                                    c0 = (wp_ * 12 + jj) * 128
                                    nc.tensor.matmul(
                                        ye_ps[:], w2m[:, c0:c0 + 128],
                                        ht_[:, jj * N:(jj + 1) * N],
                                        start=(i == 0), stop=(i == 35))
                                    i += 1
                            mos = slice(mo * N, (mo + 1) * N)
                            if dh == 0:
                                nc.vector.tensor_copy(yeh[:, mos], ye_ps[:])
                            else:
                                ysc = sm.tile([128, N], F32, tag="ycp", bufs=2)
                                nc.vector.tensor_add(ysc[:], ye_ps[:],
                                                     yeh[:, mos])
                                nc.vector.tensor_mul(ysc[:], ysc[:], cbc[:])
                                nc.sync.dma_start(
                                    ar_ins[4 * l + 2 + mo // 3][
                                        (mo % 3) * 128:(mo % 3 + 1) * 128, :],
                                    ysc[:])
                                if mo == 2:
                                    run_ar(4 * l + 2)
                    run_ar(4 * l + 3)
                else:
                    # last layer feeds no gate: single-pass fp16 (reuses the
                    # hi-split tiles so no extra SBUF), same DFF-half split
                    x1t = split3(x_sb[:], xsp, "xa", 128, KT * N, scaled=False)
                    xh11 = x1t[0]
                    hh = hsp.tile([128, 12 * N], F16, tag="hh")
                    yeh = wk.tile([128, KT * N], F32, name="yeh", bufs=1)
                    for dh in range(2):
                        for mm_ in range(12):
                            m = dh * 12 + mm_
                            w1m = w1pool.tile([128, KT * 128], F16, tag="w1s",
                                              bufs=2)
                            nc.sync.dma_start(
                                w1m[:].rearrange("q (k f) -> q k f", k=KT),
                                d_w1b[m].rearrange("k q f -> q k f"))
                            h_ps = ps_acc.tile([128, N], F32, tag="acc")
                            for k in range(KT):
                                nc.tensor.matmul(
                                    h_ps[:], w1m[:, k * 128:(k + 1) * 128],
                                    xh11[:, k * N:(k + 1) * N],
                                    start=(k == 0), stop=(k == KT - 1))
                            ht = hpool.tile([128, N], F32, tag="ht")
                            nc.scalar.activation(ht[:], h_ps[:], AF.Gelu,
                                                 bias=b1t[:, m:m + 1])
                            nc.vector.tensor_copy(
                                hh[:, mm_ * N:(mm_ + 1) * N], ht[:])
                        for mo in range(KT):
                            w2m = w2pool.tile([128, 12 * 128], F16, tag="w2s",
                                              bufs=1)
                            nc.sync.dma_start(
                                w2m[:].rearrange("q (j f) -> q j f", j=12),
                                d_w2b[mo, dh * 12:(dh + 1) * 12]
                                .rearrange("j q f -> q j f"))
                            ye_ps = ps_acc.tile([128, N], F32, tag="acc")
                            for jj in range(12):
                                nc.tensor.matmul(
                                    ye_ps[:], w2m[:, jj * 128:(jj + 1) * 128],
                                    hh[:, jj * N:(jj + 1) * N],
                                    start=(jj == 0), stop=(jj == 11))
                            mos = slice(mo * N, (mo + 1) * N)
                            if dh == 0:
                                nc.vector.tensor_copy(yeh[:, mos], ye_ps[:])
                            else:
                                ysc = sm.tile([128, N], F32, tag="ycp", bufs=2)
                                nc.vector.tensor_add(ysc[:], ye_ps[:],
                                                     yeh[:, mos])
                                nc.vector.tensor_mul(ysc[:], ysc[:], cbc[:])
                                nc.sync.dma_start(
                                    ar_ins[4 * l + 2 + mo // 3][
                                        (mo % 3) * 128:(mo % 3 + 1) * 128, :],
                                    ysc[:])
                                if mo == 2:
                                    run_ar(4 * l + 2)
                    run_ar(4 * l + 3)
                ymoe = wk.tile([128, KT * N], F32, name="yat", bufs=1)
                for k in range(KT):
                    nc.sync.dma_start(
                        ymoe[:, k * N:(k + 1) * N],
                        ar_outs[4 * l + 2 + k // 3][(k % 3) * 128:
                                                    (k % 3 + 1) * 128, :])

                # b2 contribution: sum_e comb[t,e]*b2[e,:] = b2.T @ combT
                b2c_list = []
                for k in range(KT):
                    b2c_ps = ps_bc.tile([128, N], F32, tag="bc")
                    nc.tensor.matmul(b2c_ps[:], b2t[:, k * 128:(k + 1) * 128],
                                     combT[:], start=True, stop=True)
                    b2c_list.append(b2c_ps)

                def get_t2(k, ymoe=ymoe, b2c_list=b2c_list):
                    u = sm.tile([128, N], F32, tag="res_u", bufs=2)
                    nc.vector.tensor_add(u[:], ymoe[:, k * N:(k + 1) * N],
                                         b2c_list[k][:])
                    nc.vector.tensor_add(u[:], u[:], xs(k))
                    return u[:]

                layernorm(get_t2, l2w, l2b)

            # ---- final ln + lm head ----
            def get_tf(k):
                return xs(k)

            layernorm(get_tf, lnfw, lnfb)
            lg_ps = ps_acc.tile([V, N], F32, tag="acc")
            for k in range(KT):
                nc.tensor.matmul(lg_ps[:], lmT[:, k * V:(k + 1) * V], xs(k),
                                 start=(k == 0), stop=(k == KT - 1))
            lg = sm.tile([V, N], F32, tag="lgout", bufs=1)
            nc.scalar.activation(lg[:], lg_ps[:], AF.Identity, bias=lmb[:])
            nc.sync.dma_start(d_out[:], lg[:])

    return nc


def _pair16(a, axis):
    """Stack (hi, lo*LSCALE) fp16 halves of fp32 array `a` along `axis`."""
    a = np.asarray(a, np.float32)
    hi = a.astype(np.float16)
    lo = ((a - hi.astype(np.float32)) * LSCALE).astype(np.float16)
    return np.ascontiguousarray(np.stack([hi, lo], axis=axis))


def _prep(inputs):
    """Build per-core input maps from the full input dict."""
    import ml_dtypes
    bf16 = ml_dtypes.bfloat16
    f = lambda a: np.ascontiguousarray(np.asarray(a), dtype=np.float32)
    idx = np.asarray(inputs["idx"]).reshape(1, N)
    wq, wkk, wv = f(inputs["wq"]), f(inputs["wk"]), f(inputs["wv"])
    wproj, bproj = f(inputs["wproj"]), f(inputs["bproj"])
    gate_w, gate_b = f(inputs["gate_w"]), f(inputs["gate_b"])
    w1, b1 = f(inputs["w1"]), f(inputs["b1"])
    w2, b2 = f(inputs["w2"]), f(inputs["b2"])

    base = {
        "idx": np.ascontiguousarray(idx.astype(np.int32)),
        "iota99": np.arange(V, dtype=np.float32).reshape(V, 1),
        "ident128": np.eye(128, dtype=np.float32),
        "maskb": np.where(np.tril(np.ones((64, 64), bool)), 0.0,
                          -1e30).astype(np.float32),
        "ones_col": np.ones((128, 1), np.float32),
        "ones16": np.ones((128, 1), np.float16),
        "ones_row": np.ones((1, 128), np.float32),
        "tok_emb": f(inputs["tok_emb"]),
        "posT": np.ascontiguousarray(
            np.tile(f(inputs["pos_emb"]).T, (1, B))),
        "gwT": np.ascontiguousarray(
            gate_w.transpose(0, 2, 1).reshape(L, KT, 128, E)),
        "gb": gate_b.reshape(L, 1, E),
        "b2all": b2,
        "ln1w": f(inputs["ln1_w"]).reshape(L, KT, 128),
        "ln1b": f(inputs["ln1_b"]).reshape(L, KT, 128),
        "ln2w": f(inputs["ln2_w"]).reshape(L, KT, 128),
        "ln2b": f(inputs["ln2_b"]).reshape(L, KT, 128),
        "lnfw": f(inputs["lnf_w"]).reshape(KT, 128),
        "lnfb": f(inputs["lnf_b"]).reshape(KT, 128),
        "lmT": np.ascontiguousarray(f(inputs["lm_w"]).T.reshape(KT, 128, V)),
        "lmb": f(inputs["lm_b"]).reshape(V, 1),
        "bproj": bproj.reshape(L, KT, 128),
    }
    in_maps = []
    for c in range(N_CORES):
        m = dict(base)
        m["wqp"] = _pair16(
            wq[:, c].transpose(0, 2, 1).reshape(L, KT, 128, H), 1)
        m["wkp"] = _pair16(
            wkk[:, c].transpose(0, 2, 1).reshape(L, KT, 128, H), 1)
        m["wvp"] = _pair16(
            wv[:, c].transpose(0, 2, 1).reshape(L, KT, 128, H), 1)
        m["wpp"] = _pair16(
            wproj[:, :, c * H:(c + 1) * H].transpose(0, 2, 1), 1)
        w1tc = w1[:, c].transpose(0, 2, 1)  # [L, 768, 3072]
        w1T = w1tc.reshape(L, KT, 128, MT, 128).transpose(0, 3, 1, 2, 4)
        m["w1p"] = _pair16(w1T[:L1], 2)  # [L1, MT, 2, KT, 128, 128]
        m["w1b"] = np.ascontiguousarray(w1T[L1].astype(np.float16))
        m["b1"] = np.ascontiguousarray(b1[:, c].reshape(L, MT, 128))
        w2tc = w2[:, c].transpose(0, 2, 1)  # [L, 3072, 768]
        w2T = w2tc.reshape(L, MT, 128, KT, 128).transpose(0, 3, 1, 2, 4)
        w2pp = _pair16(w2T[:L1], 2)  # [L1, KT, 2, MT, 128, 128]
        m["w2p"] = np.ascontiguousarray(
            w2pp.reshape(L1, KT, 2, 2, 12, 128, 128)
            .transpose(0, 1, 3, 2, 4, 5, 6))
        m["w2b"] = np.ascontiguousarray(w2T[L1].astype(np.float16))
        sel = np.zeros((E, 1), np.float32)
        sel[c, 0] = 1.0
        m["combsel"] = sel
        in_maps.append(m)
    return in_maps


def kernel(**inputs) -> np.ndarray:
    if "nc" not in _CACHED:
        _CACHED["nc"] = build()
    nc = _CACHED["nc"]
    in_maps = _prep(inputs)
    res = run_bass_kernel_spmd(nc, in_maps, list(range(N_CORES)))
    lt = res.results[0]["logitsT"]  # [V, N]
    return np.ascontiguousarray(lt.T.reshape(B, T, V).astype(np.float32))


if __name__ == "__main__":
    import jax

    jax.config.update("jax_platforms", "cpu")
    import reference as ref

    inp = ref.setup_inputs()
    want = np.asarray(ref.reference(**inp))
    import jax as _j
    _j.config.update("jax_platforms", "axon")
    got = kernel(**{k: np.asarray(v) for k, v in inp.items()})
    err = np.abs(got - want).max()
    rel = err / np.abs(want).max()
    l2 = np.linalg.norm(got - want) / np.linalg.norm(want)
    print(f"absmax {err:.3e}  absmax-rel {rel:.3e}  l2-rel {l2:.3e}")

